# revision 111
# baseline (speedup 1.0000x reference)
"""MDLSTM cell (2-direction) Bass/Tile kernel for Trainium2, 8-core SPMD.

Math (per direction d, with shared input projections):
    i = sigmoid(w_ii @ x + w_hi @ h_d + b_i)
    f = sigmoid(w_if @ x + w_hf @ h_d + b_f)
    g = tanh   (w_ig @ x + w_hg @ h_d + b_g)
    o = sigmoid(w_io @ x + w_ho @ h_d + b_o)
    c_d = f * c_prev_d + i * g
    h_d = o * tanh(c_d)
ct = ws0 * c_0 + ws1 * c_1 ;  ht = ws0 * h_0 + ws1 * h_1

Sharding: all activations/states split along N (=8192) across 8 cores;
weights replicated. No cross-core communication.

Per-core kernel: per output row tile (M=128) the 4 shared input
projections are computed once into PSUM (start=True groups) and copied to
SBUF; each of the 8 gate/direction accumulations then starts by injecting
that x-projection into its PSUM bank via a VectorE copy and accumulates
the hidden-projection K-tiles on top (start=False matmuls — PE-write
accumulate onto engine-written PSUM, valid because every bank's first
group in program order is a start=True group that defines has_written).
ScalarE applies sigmoid/tanh + per-partition bias straight out of PSUM;
VectorE does the elementwise cell update and direction combine.

Precision: matmul operands are bf16 except the first hidden k-tiles per
gate, which run as fp8e4m3 DoubleRow matmuls (2 k-tiles per instruction;
placed last in each accumulation group). The low-|ws| direction gets 4
fp8 k-tiles, the other 2 — its quantization error is attenuated by the
weighted sum. c_prev loads and ct/ht stores are bf16 (upcast on host).
Measured rel_fro error 1.54e-2 vs the 2e-2 gate, bit-identical to the
ml_dtypes CPU model. The direction weighted sum is folded in
algebraically: c_prev is pre-scaled by ws_d on the host, (i*ws)*g runs as
one scalar_tensor_tensor, tanh(c_d) is recovered from the weighted cw via
activation input scale 1/ws_d, and the final combine is two adds.

Startup is DMA packet-rate-bound: weights are laid out with all 4 gates
contiguous per partition row (4-8KB runs, one DMA per m-tile) and x/h
load as full tensors; weight loads at t=0 ride the Activation engine's
HW-DGE queue in parallel with Sync's x/h loads. Later weight prefetches
go via Sync only — a DMA trigger blocks its engine queue until the
previous transfer on the same hw queue completes, and the scalar queue's
ACTs free PSUM banks for the PE.
"""

import numpy as np

import concourse.bass as bass  # noqa: F401  (bass types via bacc/tile)
import concourse.mybir as mybir
import concourse.tile as tile
from concourse import bacc
from concourse.bass_utils import run_bass_kernel_spmd

N_CORES = 8
IN_C = 512
OUT_C = 1024
N = 8192
NS = N // N_CORES  # columns per core
NCH = 512  # psum free-dim chunk (one bank)
N_CHUNKS = NS // NCH
KX = IN_C // 128  # k-tiles of the input projection
KH = OUT_C // 128  # k-tiles of the hidden projection
M_TILES = OUT_C // 128

F32 = mybir.dt.float32
MM_MODE = "bf16"  # one of: "fp32r", "bf16", "fp16"
import ml_dtypes as _mld
MM_DT = {"fp32r": mybir.dt.float32r, "bf16": mybir.dt.bfloat16,
         "fp16": mybir.dt.float16}[MM_MODE]
MM_NP = {"fp32r": np.float32, "bf16": _mld.bfloat16,
         "fp16": np.float16}[MM_MODE]

SIG = mybir.ActivationFunctionType.Sigmoid
TANH = mybir.ActivationFunctionType.Tanh
MULT = mybir.AluOpType.mult
ADD = mybir.AluOpType.add
COPY = mybir.ActivationFunctionType.Copy

# The first k-tiles of each hidden projection run as fp8 DoubleRow matmuls
# (2 k-tiles per instruction at ~1.44x bf16 rate). The direction with the
# smaller |ws| gets 4 fp8 k-tiles, the other 2: its quantization error is
# attenuated by the weighted sum. Measured rel_fro 1.53e-2 (budget 2e-2);
# everything else stays bf16.
KF8_LO = 4  # fp8 k-tiles on the low-|ws| direction
KF8_HI = 2  # fp8 k-tiles on the high-|ws| direction
KF8MAX = 4
F8 = mybir.dt.float8e4
F8_NP = _mld.float8_e4m3fn
DR = mybir.MatmulPerfMode.DoubleRow


def _build(ws0: float, ws1: float):
    nc = bacc.Bacc(
        "TRN2", target_bir_lowering=False, debug=False, num_devices=N_CORES
    )

    kf8_by_dir_pre = (
        [KF8_LO, KF8_HI] if abs(ws0) <= abs(ws1) else [KF8_HI, KF8_LO]
    )
    xd = nc.dram_tensor("x", [128, KX, NS], MM_DT, kind="ExternalInput")
    # bf16 h only carries the k-tiles the bf16 path actually reads
    # (kf8..KH-1); the first k-tiles live only in the fp8 copies. A dir
    # running fully fp8 has no bf16 h at all.
    hd_ = [
        nc.dram_tensor(
            f"h{d}", [128, KH - kf8_by_dir_pre[d], NS], MM_DT,
            kind="ExternalInput",
        ) if kf8_by_dir_pre[d] < KH else None
        for d in (0, 1)
    ]
    cd_ = [
        nc.dram_tensor(f"c{d}", [OUT_C, NS], MM_DT, kind="ExternalInput")
        for d in (0, 1)
    ]
    # weights: [m_tile, partition(k%128), gate, k_tile, m_in_tile] — all 4
    # gates contiguous per partition row so one DMA moves 4-8KB runs
    # (startup is DMA packet-rate-bound, not byte-bound). bf16 wh skips
    # k-tiles 0..KF8_HI-1 (always fp8 in both directions).
    wxd = nc.dram_tensor("wx", [M_TILES, 128, 4, KX, 128], MM_DT, kind="ExternalInput")
    whd = nc.dram_tensor(
        "wh", [M_TILES, 128, 4, KH - KF8_HI, 128], MM_DT, kind="ExternalInput"
    )
    wh8d = nc.dram_tensor(
        "wh8", [M_TILES, 128, 4, KF8MAX, 128], F8, kind="ExternalInput"
    )
    kf8_by_dir = kf8_by_dir_pre
    h8d_ = [
        nc.dram_tensor(
            f"h8{d}", [128, kf8_by_dir[d], NS], F8, kind="ExternalInput"
        )
        for d in (0, 1)
    ]
    biasd = nc.dram_tensor("bias", [128, 4 * M_TILES], F32, kind="ExternalInput")
    # outputs stored bf16 (upcast on host): halves the store traffic that
    # gates the kernel-end barrier; adds only ~0.2% rms to the outputs
    ctd = nc.dram_tensor("ct", [OUT_C, NS], MM_DT, kind="ExternalOutput")
    htd = nc.dram_tensor("ht", [OUT_C, NS], MM_DT, kind="ExternalOutput")

    # reciprocal scales for the tanh(c) recovery; clamp so ws==0 stays finite
    # (then cw==0 and tanh(0)==0 gives the right answer anyway)
    inv_ws = [1.0 / max(ws0, 1e-20), 1.0 / max(ws1, 1e-20)]
    wss = [ws0, ws1]

    with tile.TileContext(nc) as tc:
        with (
            tc.tile_pool(name="resident", bufs=1) as res_pool,
            tc.tile_pool(name="wx", bufs=3) as wx_pool,
            tc.tile_pool(name="wh", bufs=3) as wh_pool,
            tc.tile_pool(name="wh8", bufs=3) as wh8_pool,
            tc.tile_pool(name="psum", bufs=8, space="PSUM") as ps_pool,
            tc.tile_pool(name="xproj", bufs=8) as xp_pool,
            tc.tile_pool(name="gates", bufs=16) as g_pool,
            tc.tile_pool(name="cprev", bufs=4) as cp_pool,
            tc.tile_pool(name="tmp", bufs=4) as t_pool,
            tc.tile_pool(name="dirres", bufs=4) as dr_pool,
            tc.tile_pool(name="out", bufs=4) as o_pool,
        ):
            wx_tiles: dict = {}
            wh_tiles: dict = {}
            wh8_tiles: dict = {}

            def load_w(mt, eng=None):
                # startup loads ride the Activation engine's HW-DGE queue so
                # the trigger path ramps in parallel with Sync's x/h loads;
                # mid-kernel prefetches go via Sync — a DMA trigger can block
                # its queue, and the scalar queue's ACTs free PSUM banks.
                eng = eng or nc.sync
                wx_tiles[mt] = wx_pool.tile(
                    [128, 4, KX, 128], MM_DT, tag="wx", name=f"wx_{mt}"
                )
                wh_tiles[mt] = wh_pool.tile(
                    [128, 4, KH - KF8_HI, 128], MM_DT, tag="wh", name=f"wh_{mt}"
                )
                wh8_tiles[mt] = wh8_pool.tile(
                    [128, 4, KF8MAX, 128], F8, tag="wh8", name=f"wh8_{mt}"
                )
                eng.dma_start(wx_tiles[mt][:], wxd[mt])
                eng.dma_start(wh_tiles[mt][:], whd[mt])
                eng.dma_start(wh8_tiles[mt][:], wh8d[mt])

            x_sb = res_pool.tile([128, KX, NS], MM_DT, tag="x")
            h_sb = [
                res_pool.tile(
                    [128, KH - kf8_by_dir[d], NS], MM_DT,
                    tag=f"h{d}", name=f"h_sb{d}",
                ) if kf8_by_dir[d] < KH else None
                for d in (0, 1)
            ]
            bias_sb = res_pool.tile([128, 4 * M_TILES], F32, tag="bias")

            # Startup is DMA packet-rate-bound: full-tensor loads give
            # 8-16KB contiguous runs (vs 1KB chunked), cutting packet count
            # ~8x. Order by first use; wx0 split per gate-pair so px g0/g1
            # can start before the full gate set lands.
            wx_tiles[0] = wx_pool.tile(
                [128, 4, KX, 128], MM_DT, tag="wx", name="wx_0"
            )
            wh_tiles[0] = wh_pool.tile(
                [128, 4, KH - KF8_HI, 128], MM_DT, tag="wh", name="wh_0"
            )
            wh8_tiles[0] = wh8_pool.tile(
                [128, 4, KF8MAX, 128], F8, tag="wh8", name="wh8_0"
            )
            nc.scalar.dma_start(wx_tiles[0][:, 0:1], wxd[0][:, 0:1])
            nc.scalar.dma_start(wx_tiles[0][:, 1:2], wxd[0][:, 1:2])
            nc.scalar.dma_start(wx_tiles[0][:, 2:4], wxd[0][:, 2:4])
            nc.scalar.dma_start(wh_tiles[0][:], whd[0])
            nc.scalar.dma_start(wh8_tiles[0][:], wh8d[0])
            h8_sb = [
                res_pool.tile(
                    [128, kf8_by_dir[d], NS], F8,
                    tag=f"h8{d}", name=f"h8_sb{d}",
                )
                for d in (0, 1)
            ]
            # x per k-tile (same layout, 2KB runs): the first px matmul
            # needs only k-tile 0 + wx gate 0 — ~0.3MB of critical startup
            # bytes before the first MM
            nc.sync.dma_start(x_sb[:, 0:1], xd[:, 0:1])
            nc.sync.dma_start(x_sb[:, 1:2], xd[:, 1:2])
            nc.sync.dma_start(x_sb[:, 2:3], xd[:, 2:3])
            nc.sync.dma_start(x_sb[:, 3:4], xd[:, 3:4])
            nc.sync.dma_start(bias_sb[:], biasd[:])
            for d in (0, 1):
                if h_sb[d] is not None:
                    nc.sync.dma_start(h_sb[d][:], hd_[d][:])
                nc.sync.dma_start(h8_sb[d][:], h8d_[d][:])
            # NOT on the scalar queue: a DMA trigger blocks its queue until
            # the previous transfer on the same hw queue completes, and the
            # scalar queue must stay free for the px copies the dir phases
            # wait on.
            load_w(1)

            def px_phase(mt, n, wxm):
                nsl = slice(n * NCH, (n + 1) * NCH)
                xp = []
                for g in range(4):
                    px = ps_pool.tile(
                        [128, NCH], F32, tag="ps", name=f"px_{mt}_{n}_{g}"
                    )
                    for kt in range(KX):
                        nc.tensor.matmul(
                            px[:],
                            wxm[:, g, kt, :],
                            x_sb[:, kt, nsl],
                            start=(kt == 0),
                            stop=(kt == KX - 1),
                        )
                    xpt = xp_pool.tile(
                        [128, NCH], F32, tag="xp", name=f"xp_{mt}_{n}_{g}"
                    )
                    nc.scalar.activation(xpt[:], px[:], COPY)
                    xp.append(xpt)
                return xp

            def dir_phase(mt, n, d, xp, whm, wh8m, msl):
                # All weighted-sum algebra is folded in: cw = ws_d * c_d via
                # host-prescaled c_prev and the (i*ws)*g trick; tanh(c_d) is
                # recovered from cw with the activation input scale 1/ws_d;
                # hw = (o*ws)*tanh(c_d). The combine is then just two adds.
                nsl = slice(n * NCH, (n + 1) * NCH)
                ws, iws = wss[d], inv_ws[d]
                gt = []
                for g in range(4):
                    ps = ps_pool.tile(
                        [128, NCH], F32, tag="ps", name=f"ps_{mt}_{n}_{d}_{g}"
                    )
                    # inject the shared x-projection, then accumulate the
                    # hidden projection on top of it (PE-write accumulate on
                    # engine-written PSUM; every bank's first group in
                    # program order is a start=True px group)
                    kf8 = kf8_by_dir[d]
                    nc.vector.tensor_copy(ps[:], xp[g][:])
                    for kh in range(kf8, KH):
                        nc.tensor.matmul(
                            ps[:],
                            whm[:, g, kh - KF8_HI, :],
                            h_sb[d][:, kh - kf8, nsl],
                            start=False,
                            stop=False,
                            skip_group_check=True,
                        )
                    # k-tiles 0..kf8-1 as fp8 DoubleRow pairs, last in the
                    # group so their operands aren't needed at group start
                    for p in range(kf8 // 2):
                        nc.tensor.matmul(
                            ps[:],
                            wh8m[:, g, 2 * p : 2 * p + 2, :],
                            h8_sb[d][:, 2 * p : 2 * p + 2, nsl],
                            start=False,
                            stop=(p == kf8 // 2 - 1),
                            skip_group_check=True,
                            perf_mode=DR,
                        )
                    gact = g_pool.tile(
                        [128, NCH], F32, tag="gate", name=f"gate_{mt}_{n}_{d}_{g}"
                    )
                    nc.scalar.activation(
                        gact[:],
                        ps[:],
                        TANH if g == 2 else SIG,
                        bias=bias_sb[:, g * M_TILES + mt : g * M_TILES + mt + 1],
                    )
                    gt.append(gact)

                # Plain muls run on GPSIMD (it rejects scalar_tensor_tensor)
                # so the vector queue stays shallow: PSUM-bank-freeing
                # pre-adds must not sit behind elementwise ops (in-order
                # engine queues). The ws scaling folds into the vector stt.
                cp = cp_pool.tile([128, NCH], MM_DT, tag="cp")
                nc.sync.dma_start(cp[:], cd_[d][msl, nsl])  # ws_d * c_prev_d
                ig = t_pool.tile([128, NCH], F32, tag="ig")
                nc.vector.scalar_tensor_tensor(ig[:], gt[0][:], ws, gt[2][:], MULT, MULT)
                fc = t_pool.tile([128, NCH], F32, tag="fc")
                nc.vector.tensor_mul(fc[:], gt[1][:], cp[:])
                cw = dr_pool.tile([128, NCH], F32, tag="cw")
                nc.vector.tensor_add(cw[:], ig[:], fc[:])
                tch = t_pool.tile([128, NCH], F32, tag="tch")
                nc.scalar.activation(tch[:], cw[:], TANH, scale=iws)
                hw = dr_pool.tile([128, NCH], F32, tag="hw")
                nc.vector.scalar_tensor_tensor(hw[:], gt[3][:], ws, tch[:], MULT, MULT)
                return cw, hw

            def combine(n, msl, cdir, hdir):
                nsl = slice(n * NCH, (n + 1) * NCH)
                # output triggers ride the scalar queue: sync is busy with
                # cp loads + weight prefetches and the final htt store gates
                # the kernel end
                ctt = o_pool.tile([128, NCH], MM_DT, tag="ctt")
                nc.vector.tensor_add(ctt[:], cdir[0][:], cdir[1][:])
                nc.scalar.dma_start(ctd[msl, nsl], ctt[:])
                htt = o_pool.tile([128, NCH], MM_DT, tag="htt")
                nc.vector.tensor_add(htt[:], hdir[0][:], hdir[1][:])
                nc.scalar.dma_start(htd[msl, nsl], htt[:])

            for mt in range(M_TILES):
                msl = slice(mt * 128, (mt + 1) * 128)
                if mt + 2 < M_TILES:
                    load_w(mt + 2)
                wxm = wx_tiles.pop(mt)
                whm = wh_tiles.pop(mt)
                wh8m = wh8_tiles.pop(mt)

                # Both n-chunks' input projections first: at kernel start
                # these 8 start=True groups cover all 8 PSUM banks, so no
                # inject group ever runs on a virgin bank with undefined
                # has_written (accumulate-vs-overwrite) state.
                xp0 = px_phase(mt, 0, wxm)
                xp1 = px_phase(mt, 1, wxm)
                c00, h00 = dir_phase(mt, 0, 0, xp0, whm, wh8m, msl)
                c10, h10 = dir_phase(mt, 0, 1, xp0, whm, wh8m, msl)
                combine(0, msl, [c00, c10], [h00, h10])
                c01, h01 = dir_phase(mt, 1, 0, xp1, whm, wh8m, msl)
                c11, h11 = dir_phase(mt, 1, 1, xp1, whm, wh8m, msl)
                combine(1, msl, [c01, c11], [h01, h11])

    nc.finalize()
    n_mm = sum(
        1 for i in nc.inst_map.values() if type(i).__name__ == "InstMatmult"
    )
    expected_mm = M_TILES * N_CHUNKS * 4 * (
        KX + sum(KH - kf8 + kf8 // 2 for kf8 in kf8_by_dir)
    )
    assert n_mm == expected_mm, f"matmul count {n_mm} != {expected_mm}"
    return nc


_CACHE: dict = {}


def _get_nc(ws0: float, ws1: float):
    key = (ws0, ws1)
    if key not in _CACHE:
        _CACHE.clear()
        _CACHE[key] = _build(ws0, ws1)
    return _CACHE[key]


def _prep_w(w: np.ndarray, kt: int) -> np.ndarray:
    """(OUT_C, K) weight -> [m_tile, partition, k_tile, m_in_tile] lhsT tiles."""
    wT = np.ascontiguousarray(w.T)  # (K, OUT_C)
    k = wT.shape[0]
    assert k == kt * 128
    r = wT.reshape(kt, 128, M_TILES, 128)  # [ktile, p, mtile, mi]
    return np.ascontiguousarray(r.transpose(2, 1, 0, 3).astype(MM_NP))


def _prep_wstack(ws: list[np.ndarray], kt: int) -> np.ndarray:
    """4 gate weights -> [m_tile, partition, gate, k_tile, m_in_tile]."""
    s = np.stack([_prep_w(w, kt) for w in ws])  # [g, mt, p, kt, mi]
    return np.ascontiguousarray(s.transpose(1, 2, 0, 3, 4))


def _prep_wstack8(ws: list[np.ndarray]) -> np.ndarray:
    """First KF8MAX k-tiles of the 4 hidden weights, as fp8 lhsT pairs."""
    tiles = []
    for w in ws:  # (OUT_C, K)
        wT = np.ascontiguousarray(w.T[: KF8MAX * 128])  # (KF8MAX*128, OUT_C)
        r = wT.reshape(KF8MAX, 128, M_TILES, 128)
        tiles.append(r.transpose(2, 1, 0, 3))  # [mt, p, kt, mi]
    s = np.stack(tiles)  # [g, mt, p, kt, mi]
    return np.ascontiguousarray(s.transpose(1, 2, 0, 3, 4).astype(F8_NP))


def _prep_rhs(a: np.ndarray, kt: int) -> np.ndarray:
    """(K, n) activation -> [partition, k_tile, n]."""
    k, n = a.shape
    assert k == kt * 128
    return np.ascontiguousarray(a.reshape(kt, 128, n).transpose(1, 0, 2).astype(MM_NP))


def _prep_rhs8(a: np.ndarray, kt: int) -> np.ndarray:
    """(kt*128, n) activation -> fp8 [partition, k_tile, n] (direct cast)."""
    k, n = a.shape
    r = a.reshape(kt, 128, n).transpose(1, 0, 2)
    return np.ascontiguousarray(r.astype(F8_NP))


def run(inputs: dict, trace: bool = False, trace_kwargs: dict | None = None):
    x = np.asarray(inputs["x"], dtype=np.float32)
    ws = np.asarray(inputs["weighted_sum"], dtype=np.float32)
    ws0, ws1 = float(ws[0]), float(ws[1])
    nc = _get_nc(ws0, ws1)

    wx_host = _prep_wstack(
        [np.asarray(inputs[k], dtype=np.float32)
         for k in ("w_ii", "w_if", "w_ig", "w_io")], KX
    )
    wh_list = [np.asarray(inputs[k], dtype=np.float32)
               for k in ("w_hi", "w_hf", "w_hg", "w_ho")]
    wh_host = _prep_wstack(
        [w[:, KF8_HI * 128 :] for w in wh_list], KH - KF8_HI
    )
    wh8_host = _prep_wstack8(wh_list)
    kf8_by_dir = [KF8_LO, KF8_HI] if abs(ws0) <= abs(ws1) else [KF8_HI, KF8_LO]
    bias_host = np.concatenate(
        [np.asarray(inputs[k], dtype=np.float32).reshape(M_TILES, 128).T
         for k in ("b_i", "b_f", "b_g", "b_o")],
        axis=1,
    )
    bias_host = np.ascontiguousarray(bias_host)

    h0 = np.asarray(inputs["h_prev_dim0"], dtype=np.float32)
    h1 = np.asarray(inputs["h_prev_dim1"], dtype=np.float32)
    # c_prev is pre-scaled by the direction weight on the host; the kernel
    # computes cw_d = ws_d*c_d directly and ct = cw_0 + cw_1.
    c0 = (np.asarray(inputs["c_prev_dim0"], dtype=np.float32) * ws0).astype(MM_NP)
    c1 = (np.asarray(inputs["c_prev_dim1"], dtype=np.float32) * ws1).astype(MM_NP)

    in_maps = []
    for core in range(N_CORES):
        csl = slice(core * NS, (core + 1) * NS)
        m = {
            "x": _prep_rhs(x[:, csl], KX),
            "h80": _prep_rhs8(h0[: kf8_by_dir[0] * 128, csl], kf8_by_dir[0]),
            "h81": _prep_rhs8(h1[: kf8_by_dir[1] * 128, csl], kf8_by_dir[1]),
            "c0": np.ascontiguousarray(c0[:, csl]),
            "c1": np.ascontiguousarray(c1[:, csl]),
            "wx": wx_host,
            "wh": wh_host,
            "wh8": wh8_host,
            "bias": bias_host,
        }
        for d, h in ((0, h0), (1, h1)):
            if kf8_by_dir[d] < KH:
                m[f"h{d}"] = _prep_rhs(
                    h[kf8_by_dir[d] * 128 :, csl], KH - kf8_by_dir[d]
                )
        in_maps.append(m)

    res = run_bass_kernel_spmd(
        nc,
        in_maps,
        list(range(N_CORES)),
        trace=trace,
        **(trace_kwargs or {}),
    )
    ct = np.concatenate(
        [res.results[c]["ct"].astype(np.float32) for c in range(N_CORES)], axis=1
    )
    ht = np.concatenate(
        [res.results[c]["ht"].astype(np.float32) for c in range(N_CORES)], axis=1
    )
    return (ct, ht), res


def kernel(**inputs) -> tuple:
    (ct, ht), _ = run(inputs)
    return ct, ht



# revision 113
# speedup vs baseline: 1.0114x; 1.0114x over previous
"""MDLSTM cell (2-direction) Bass/Tile kernel for Trainium2, 8-core SPMD.

Math (per direction d, with shared input projections):
    i = sigmoid(w_ii @ x + w_hi @ h_d + b_i)
    f = sigmoid(w_if @ x + w_hf @ h_d + b_f)
    g = tanh   (w_ig @ x + w_hg @ h_d + b_g)
    o = sigmoid(w_io @ x + w_ho @ h_d + b_o)
    c_d = f * c_prev_d + i * g
    h_d = o * tanh(c_d)
ct = ws0 * c_0 + ws1 * c_1 ;  ht = ws0 * h_0 + ws1 * h_1

Sharding: all activations/states split along N (=8192) across 8 cores;
weights replicated. No cross-core communication.

Per-core kernel: per output row tile (M=128) the 4 shared input
projections are computed once into PSUM (start=True groups) and copied to
SBUF; each of the 8 gate/direction accumulations then starts by injecting
that x-projection into its PSUM bank via a VectorE copy and accumulates
the hidden-projection K-tiles on top (start=False matmuls — PE-write
accumulate onto engine-written PSUM, valid because every bank's first
group in program order is a start=True group that defines has_written).
ScalarE applies sigmoid/tanh + per-partition bias straight out of PSUM;
VectorE does the elementwise cell update and direction combine.

Precision: matmul operands are bf16 except the first hidden k-tiles per
gate, which run as fp8e4m3 DoubleRow matmuls (2 k-tiles per instruction;
placed last in each accumulation group). The low-|ws| direction gets 4
fp8 k-tiles, the other 2 — its quantization error is attenuated by the
weighted sum. c_prev loads and ct/ht stores are bf16 (upcast on host).
Measured rel_fro error 1.54e-2 vs the 2e-2 gate, bit-identical to the
ml_dtypes CPU model. The direction weighted sum is folded in
algebraically: c_prev is pre-scaled by ws_d on the host, (i*ws)*g runs as
one scalar_tensor_tensor, tanh(c_d) is recovered from the weighted cw via
activation input scale 1/ws_d, and the final combine is two adds.

Startup is DMA packet-rate-bound: weights are laid out with all 4 gates
contiguous per partition row (4-8KB runs, one DMA per m-tile) and x/h
load as full tensors; weight loads at t=0 ride the Activation engine's
HW-DGE queue in parallel with Sync's x/h loads. Later weight prefetches
go via Sync only — a DMA trigger blocks its engine queue until the
previous transfer on the same hw queue completes, and the scalar queue's
ACTs free PSUM banks for the PE.
"""

import numpy as np

import concourse.bass as bass  # noqa: F401  (bass types via bacc/tile)
import concourse.mybir as mybir
import concourse.tile as tile
from concourse import bacc
from concourse.bass_utils import run_bass_kernel_spmd

N_CORES = 8
IN_C = 512
OUT_C = 1024
N = 8192
NS = N // N_CORES  # columns per core
NCH = 512  # psum free-dim chunk (one bank)
N_CHUNKS = NS // NCH
KX = IN_C // 128  # k-tiles of the input projection
KH = OUT_C // 128  # k-tiles of the hidden projection
M_TILES = OUT_C // 128

F32 = mybir.dt.float32
MM_MODE = "bf16"  # one of: "fp32r", "bf16", "fp16"
import ml_dtypes as _mld
MM_DT = {"fp32r": mybir.dt.float32r, "bf16": mybir.dt.bfloat16,
         "fp16": mybir.dt.float16}[MM_MODE]
MM_NP = {"fp32r": np.float32, "bf16": _mld.bfloat16,
         "fp16": np.float16}[MM_MODE]

SIG = mybir.ActivationFunctionType.Sigmoid
TANH = mybir.ActivationFunctionType.Tanh
MULT = mybir.AluOpType.mult
ADD = mybir.AluOpType.add
COPY = mybir.ActivationFunctionType.Copy

# The first k-tiles of each hidden projection run as fp8 DoubleRow matmuls
# (2 k-tiles per instruction at ~1.44x bf16 rate). The direction with the
# smaller |ws| gets 4 fp8 k-tiles, the other 2: its quantization error is
# attenuated by the weighted sum. Measured rel_fro 1.53e-2 (budget 2e-2);
# everything else stays bf16.
KF8_LO = 4  # fp8 k-tiles on the low-|ws| direction
KF8_HI = 2  # fp8 k-tiles on the high-|ws| direction
KF8MAX = 4
F8 = mybir.dt.float8e4
F8_NP = _mld.float8_e4m3fn
DR = mybir.MatmulPerfMode.DoubleRow


def _build(ws0: float, ws1: float):
    nc = bacc.Bacc(
        "TRN2", target_bir_lowering=False, debug=False, num_devices=N_CORES
    )

    kf8_by_dir_pre = (
        [KF8_LO, KF8_HI] if abs(ws0) <= abs(ws1) else [KF8_HI, KF8_LO]
    )
    xd = nc.dram_tensor("x", [128, KX, NS], MM_DT, kind="ExternalInput")
    # bf16 h only carries the k-tiles the bf16 path actually reads
    # (kf8..KH-1); the first k-tiles live only in the fp8 copies. A dir
    # running fully fp8 has no bf16 h at all.
    hd_ = [
        nc.dram_tensor(
            f"h{d}", [128, KH - kf8_by_dir_pre[d], NS], MM_DT,
            kind="ExternalInput",
        ) if kf8_by_dir_pre[d] < KH else None
        for d in (0, 1)
    ]
    cd_ = [
        nc.dram_tensor(f"c{d}", [OUT_C, NS], MM_DT, kind="ExternalInput")
        for d in (0, 1)
    ]
    # weights: [m_tile, partition(k%128), gate, k_tile, m_in_tile] — all 4
    # gates contiguous per partition row so one DMA moves 4-8KB runs
    # (startup is DMA packet-rate-bound, not byte-bound). bf16 wh skips
    # k-tiles 0..KF8_HI-1 (always fp8 in both directions).
    wxd = nc.dram_tensor("wx", [M_TILES, 128, 4, KX, 128], MM_DT, kind="ExternalInput")
    whd = nc.dram_tensor(
        "wh", [M_TILES, 128, 4, KH - KF8_HI, 128], MM_DT, kind="ExternalInput"
    )
    wh8d = nc.dram_tensor(
        "wh8", [M_TILES, 128, 4, KF8MAX, 128], F8, kind="ExternalInput"
    )
    kf8_by_dir = kf8_by_dir_pre
    h8d_ = [
        nc.dram_tensor(
            f"h8{d}", [128, kf8_by_dir[d], NS], F8, kind="ExternalInput"
        )
        for d in (0, 1)
    ]
    biasd = nc.dram_tensor("bias", [128, 4 * M_TILES], F32, kind="ExternalInput")
    # outputs stored bf16 (upcast on host): halves the store traffic that
    # gates the kernel-end barrier; adds only ~0.2% rms to the outputs
    ctd = nc.dram_tensor("ct", [OUT_C, NS], MM_DT, kind="ExternalOutput")
    htd = nc.dram_tensor("ht", [OUT_C, NS], MM_DT, kind="ExternalOutput")

    # reciprocal scales for the tanh(c) recovery; clamp so ws==0 stays finite
    # (then cw==0 and tanh(0)==0 gives the right answer anyway)
    inv_ws = [1.0 / max(ws0, 1e-20), 1.0 / max(ws1, 1e-20)]
    wss = [ws0, ws1]

    with tile.TileContext(nc) as tc:
        with (
            tc.tile_pool(name="resident", bufs=1) as res_pool,
            tc.tile_pool(name="wx", bufs=3) as wx_pool,
            tc.tile_pool(name="wh", bufs=3) as wh_pool,
            tc.tile_pool(name="wh8", bufs=3) as wh8_pool,
            tc.tile_pool(name="psum", bufs=8, space="PSUM") as ps_pool,
            tc.tile_pool(name="xproj", bufs=8) as xp_pool,
            tc.tile_pool(name="gates", bufs=16) as g_pool,
            tc.tile_pool(name="cprev", bufs=4) as cp_pool,
            tc.tile_pool(name="tmp", bufs=4) as t_pool,
            tc.tile_pool(name="dirres", bufs=4) as dr_pool,
            tc.tile_pool(name="out", bufs=4) as o_pool,
        ):
            wx_tiles: dict = {}
            wh_tiles: dict = {}
            wh8_tiles: dict = {}

            def load_w(mt, eng=None):
                # startup loads ride the Activation engine's HW-DGE queue so
                # the trigger path ramps in parallel with Sync's x/h loads;
                # mid-kernel prefetches go via Sync — a DMA trigger can block
                # its queue, and the scalar queue's ACTs free PSUM banks.
                eng = eng or nc.sync
                wx_tiles[mt] = wx_pool.tile(
                    [128, 4, KX, 128], MM_DT, tag="wx", name=f"wx_{mt}"
                )
                wh_tiles[mt] = wh_pool.tile(
                    [128, 4, KH - KF8_HI, 128], MM_DT, tag="wh", name=f"wh_{mt}"
                )
                wh8_tiles[mt] = wh8_pool.tile(
                    [128, 4, KF8MAX, 128], F8, tag="wh8", name=f"wh8_{mt}"
                )
                eng.dma_start(wx_tiles[mt][:], wxd[mt])
                eng.dma_start(wh_tiles[mt][:], whd[mt])
                eng.dma_start(wh8_tiles[mt][:], wh8d[mt])

            x_sb = res_pool.tile([128, KX, NS], MM_DT, tag="x")
            h_sb = [
                res_pool.tile(
                    [128, KH - kf8_by_dir[d], NS], MM_DT,
                    tag=f"h{d}", name=f"h_sb{d}",
                ) if kf8_by_dir[d] < KH else None
                for d in (0, 1)
            ]
            bias_sb = res_pool.tile([128, 4 * M_TILES], F32, tag="bias")

            # Startup is DMA packet-rate-bound: full-tensor loads give
            # 8-16KB contiguous runs (vs 1KB chunked), cutting packet count
            # ~8x. Order by first use; wx0 split per gate-pair so px g0/g1
            # can start before the full gate set lands.
            wx_tiles[0] = wx_pool.tile(
                [128, 4, KX, 128], MM_DT, tag="wx", name="wx_0"
            )
            wh_tiles[0] = wh_pool.tile(
                [128, 4, KH - KF8_HI, 128], MM_DT, tag="wh", name="wh_0"
            )
            wh8_tiles[0] = wh8_pool.tile(
                [128, 4, KF8MAX, 128], F8, tag="wh8", name="wh8_0"
            )
            nc.scalar.dma_start(wx_tiles[0][:, 0:2], wxd[0][:, 0:2])
            nc.scalar.dma_start(wx_tiles[0][:, 2:4], wxd[0][:, 2:4])
            nc.scalar.dma_start(wh_tiles[0][:], whd[0])
            nc.scalar.dma_start(wh8_tiles[0][:], wh8d[0])
            h8_sb = [
                res_pool.tile(
                    [128, kf8_by_dir[d], NS], F8,
                    tag=f"h8{d}", name=f"h8_sb{d}",
                )
                for d in (0, 1)
            ]
            # x in two k-tile halves (same layout, 4KB runs): the first px
            # matmuls only need k-tiles 0-1, shaving the critical startup
            # bytes before the first MM
            nc.sync.dma_start(x_sb[:, 0:2], xd[:, 0:2])
            nc.sync.dma_start(x_sb[:, 2:4], xd[:, 2:4])
            nc.sync.dma_start(bias_sb[:], biasd[:])
            for d in (0, 1):
                if h_sb[d] is not None:
                    nc.sync.dma_start(h_sb[d][:], hd_[d][:])
                nc.sync.dma_start(h8_sb[d][:], h8d_[d][:])
            # NOT on the scalar queue: a DMA trigger blocks its queue until
            # the previous transfer on the same hw queue completes, and the
            # scalar queue must stay free for the px copies the dir phases
            # wait on.
            load_w(1)

            def px_phase(mt, n, wxm):
                nsl = slice(n * NCH, (n + 1) * NCH)
                xp = []
                for g in range(4):
                    px = ps_pool.tile(
                        [128, NCH], F32, tag="ps", name=f"px_{mt}_{n}_{g}"
                    )
                    for kt in range(KX):
                        nc.tensor.matmul(
                            px[:],
                            wxm[:, g, kt, :],
                            x_sb[:, kt, nsl],
                            start=(kt == 0),
                            stop=(kt == KX - 1),
                        )
                    xpt = xp_pool.tile(
                        [128, NCH], F32, tag="xp", name=f"xp_{mt}_{n}_{g}"
                    )
                    nc.scalar.activation(xpt[:], px[:], COPY)
                    xp.append(xpt)
                return xp

            def dir_phase(mt, n, d, xp, whm, wh8m, msl):
                # All weighted-sum algebra is folded in: cw = ws_d * c_d via
                # host-prescaled c_prev and the (i*ws)*g trick; tanh(c_d) is
                # recovered from cw with the activation input scale 1/ws_d;
                # hw = (o*ws)*tanh(c_d). The combine is then just two adds.
                nsl = slice(n * NCH, (n + 1) * NCH)
                ws, iws = wss[d], inv_ws[d]
                gt = []
                for g in range(4):
                    ps = ps_pool.tile(
                        [128, NCH], F32, tag="ps", name=f"ps_{mt}_{n}_{d}_{g}"
                    )
                    # inject the shared x-projection, then accumulate the
                    # hidden projection on top of it (PE-write accumulate on
                    # engine-written PSUM; every bank's first group in
                    # program order is a start=True px group)
                    kf8 = kf8_by_dir[d]
                    nc.vector.tensor_copy(ps[:], xp[g][:])
                    for kh in range(kf8, KH):
                        nc.tensor.matmul(
                            ps[:],
                            whm[:, g, kh - KF8_HI, :],
                            h_sb[d][:, kh - kf8, nsl],
                            start=False,
                            stop=False,
                            skip_group_check=True,
                        )
                    # k-tiles 0..kf8-1 as fp8 DoubleRow pairs, last in the
                    # group so their operands aren't needed at group start
                    for p in range(kf8 // 2):
                        nc.tensor.matmul(
                            ps[:],
                            wh8m[:, g, 2 * p : 2 * p + 2, :],
                            h8_sb[d][:, 2 * p : 2 * p + 2, nsl],
                            start=False,
                            stop=(p == kf8 // 2 - 1),
                            skip_group_check=True,
                            perf_mode=DR,
                        )
                    gact = g_pool.tile(
                        [128, NCH], F32, tag="gate", name=f"gate_{mt}_{n}_{d}_{g}"
                    )
                    nc.scalar.activation(
                        gact[:],
                        ps[:],
                        TANH if g == 2 else SIG,
                        bias=bias_sb[:, g * M_TILES + mt : g * M_TILES + mt + 1],
                    )
                    gt.append(gact)

                # Plain muls run on GPSIMD (it rejects scalar_tensor_tensor)
                # so the vector queue stays shallow: PSUM-bank-freeing
                # pre-adds must not sit behind elementwise ops (in-order
                # engine queues). The ws scaling folds into the vector stt.
                cp = cp_pool.tile([128, NCH], MM_DT, tag="cp")
                nc.sync.dma_start(cp[:], cd_[d][msl, nsl])  # ws_d * c_prev_d
                ig = t_pool.tile([128, NCH], F32, tag="ig")
                nc.vector.scalar_tensor_tensor(ig[:], gt[0][:], ws, gt[2][:], MULT, MULT)
                fc = t_pool.tile([128, NCH], F32, tag="fc")
                nc.vector.tensor_mul(fc[:], gt[1][:], cp[:])
                cw = dr_pool.tile([128, NCH], F32, tag="cw")
                nc.vector.tensor_add(cw[:], ig[:], fc[:])
                tch = t_pool.tile([128, NCH], F32, tag="tch")
                nc.scalar.activation(tch[:], cw[:], TANH, scale=iws)
                hw = dr_pool.tile([128, NCH], F32, tag="hw")
                nc.vector.scalar_tensor_tensor(hw[:], gt[3][:], ws, tch[:], MULT, MULT)
                return cw, hw

            def combine(n, msl, cdir, hdir):
                nsl = slice(n * NCH, (n + 1) * NCH)
                # output triggers ride the scalar queue: sync is busy with
                # cp loads + weight prefetches and the final htt store gates
                # the kernel end
                ctt = o_pool.tile([128, NCH], MM_DT, tag="ctt")
                nc.vector.tensor_add(ctt[:], cdir[0][:], cdir[1][:])
                nc.scalar.dma_start(ctd[msl, nsl], ctt[:])
                htt = o_pool.tile([128, NCH], MM_DT, tag="htt")
                nc.vector.tensor_add(htt[:], hdir[0][:], hdir[1][:])
                nc.scalar.dma_start(htd[msl, nsl], htt[:])

            for mt in range(M_TILES):
                msl = slice(mt * 128, (mt + 1) * 128)
                if mt + 2 < M_TILES:
                    load_w(mt + 2)
                wxm = wx_tiles.pop(mt)
                whm = wh_tiles.pop(mt)
                wh8m = wh8_tiles.pop(mt)

                # Both n-chunks' input projections first: at kernel start
                # these 8 start=True groups cover all 8 PSUM banks, so no
                # inject group ever runs on a virgin bank with undefined
                # has_written (accumulate-vs-overwrite) state.
                xp0 = px_phase(mt, 0, wxm)
                xp1 = px_phase(mt, 1, wxm)
                c00, h00 = dir_phase(mt, 0, 0, xp0, whm, wh8m, msl)
                c10, h10 = dir_phase(mt, 0, 1, xp0, whm, wh8m, msl)
                combine(0, msl, [c00, c10], [h00, h10])
                c01, h01 = dir_phase(mt, 1, 0, xp1, whm, wh8m, msl)
                c11, h11 = dir_phase(mt, 1, 1, xp1, whm, wh8m, msl)
                combine(1, msl, [c01, c11], [h01, h11])

    nc.finalize()
    n_mm = sum(
        1 for i in nc.inst_map.values() if type(i).__name__ == "InstMatmult"
    )
    expected_mm = M_TILES * N_CHUNKS * 4 * (
        KX + sum(KH - kf8 + kf8 // 2 for kf8 in kf8_by_dir)
    )
    assert n_mm == expected_mm, f"matmul count {n_mm} != {expected_mm}"
    return nc


_CACHE: dict = {}


def _get_nc(ws0: float, ws1: float):
    key = (ws0, ws1)
    if key not in _CACHE:
        _CACHE.clear()
        _CACHE[key] = _build(ws0, ws1)
    return _CACHE[key]


def _prep_w(w: np.ndarray, kt: int) -> np.ndarray:
    """(OUT_C, K) weight -> [m_tile, partition, k_tile, m_in_tile] lhsT tiles."""
    wT = np.ascontiguousarray(w.T)  # (K, OUT_C)
    k = wT.shape[0]
    assert k == kt * 128
    r = wT.reshape(kt, 128, M_TILES, 128)  # [ktile, p, mtile, mi]
    return np.ascontiguousarray(r.transpose(2, 1, 0, 3).astype(MM_NP))


def _prep_wstack(ws: list[np.ndarray], kt: int) -> np.ndarray:
    """4 gate weights -> [m_tile, partition, gate, k_tile, m_in_tile]."""
    s = np.stack([_prep_w(w, kt) for w in ws])  # [g, mt, p, kt, mi]
    return np.ascontiguousarray(s.transpose(1, 2, 0, 3, 4))


def _prep_wstack8(ws: list[np.ndarray]) -> np.ndarray:
    """First KF8MAX k-tiles of the 4 hidden weights, as fp8 lhsT pairs."""
    tiles = []
    for w in ws:  # (OUT_C, K)
        wT = np.ascontiguousarray(w.T[: KF8MAX * 128])  # (KF8MAX*128, OUT_C)
        r = wT.reshape(KF8MAX, 128, M_TILES, 128)
        tiles.append(r.transpose(2, 1, 0, 3))  # [mt, p, kt, mi]
    s = np.stack(tiles)  # [g, mt, p, kt, mi]
    return np.ascontiguousarray(s.transpose(1, 2, 0, 3, 4).astype(F8_NP))


def _prep_rhs(a: np.ndarray, kt: int) -> np.ndarray:
    """(K, n) activation -> [partition, k_tile, n]."""
    k, n = a.shape
    assert k == kt * 128
    return np.ascontiguousarray(a.reshape(kt, 128, n).transpose(1, 0, 2).astype(MM_NP))


def _prep_rhs8(a: np.ndarray, kt: int) -> np.ndarray:
    """(kt*128, n) activation -> fp8 [partition, k_tile, n] (direct cast)."""
    k, n = a.shape
    r = a.reshape(kt, 128, n).transpose(1, 0, 2)
    return np.ascontiguousarray(r.astype(F8_NP))


def run(inputs: dict, trace: bool = False, trace_kwargs: dict | None = None):
    x = np.asarray(inputs["x"], dtype=np.float32)
    ws = np.asarray(inputs["weighted_sum"], dtype=np.float32)
    ws0, ws1 = float(ws[0]), float(ws[1])
    nc = _get_nc(ws0, ws1)

    wx_host = _prep_wstack(
        [np.asarray(inputs[k], dtype=np.float32)
         for k in ("w_ii", "w_if", "w_ig", "w_io")], KX
    )
    wh_list = [np.asarray(inputs[k], dtype=np.float32)
               for k in ("w_hi", "w_hf", "w_hg", "w_ho")]
    wh_host = _prep_wstack(
        [w[:, KF8_HI * 128 :] for w in wh_list], KH - KF8_HI
    )
    wh8_host = _prep_wstack8(wh_list)
    kf8_by_dir = [KF8_LO, KF8_HI] if abs(ws0) <= abs(ws1) else [KF8_HI, KF8_LO]
    bias_host = np.concatenate(
        [np.asarray(inputs[k], dtype=np.float32).reshape(M_TILES, 128).T
         for k in ("b_i", "b_f", "b_g", "b_o")],
        axis=1,
    )
    bias_host = np.ascontiguousarray(bias_host)

    h0 = np.asarray(inputs["h_prev_dim0"], dtype=np.float32)
    h1 = np.asarray(inputs["h_prev_dim1"], dtype=np.float32)
    # c_prev is pre-scaled by the direction weight on the host; the kernel
    # computes cw_d = ws_d*c_d directly and ct = cw_0 + cw_1.
    c0 = (np.asarray(inputs["c_prev_dim0"], dtype=np.float32) * ws0).astype(MM_NP)
    c1 = (np.asarray(inputs["c_prev_dim1"], dtype=np.float32) * ws1).astype(MM_NP)

    in_maps = []
    for core in range(N_CORES):
        csl = slice(core * NS, (core + 1) * NS)
        m = {
            "x": _prep_rhs(x[:, csl], KX),
            "h80": _prep_rhs8(h0[: kf8_by_dir[0] * 128, csl], kf8_by_dir[0]),
            "h81": _prep_rhs8(h1[: kf8_by_dir[1] * 128, csl], kf8_by_dir[1]),
            "c0": np.ascontiguousarray(c0[:, csl]),
            "c1": np.ascontiguousarray(c1[:, csl]),
            "wx": wx_host,
            "wh": wh_host,
            "wh8": wh8_host,
            "bias": bias_host,
        }
        for d, h in ((0, h0), (1, h1)):
            if kf8_by_dir[d] < KH:
                m[f"h{d}"] = _prep_rhs(
                    h[kf8_by_dir[d] * 128 :, csl], KH - kf8_by_dir[d]
                )
        in_maps.append(m)

    res = run_bass_kernel_spmd(
        nc,
        in_maps,
        list(range(N_CORES)),
        trace=trace,
        **(trace_kwargs or {}),
    )
    ct = np.concatenate(
        [res.results[c]["ct"].astype(np.float32) for c in range(N_CORES)], axis=1
    )
    ht = np.concatenate(
        [res.results[c]["ht"].astype(np.float32) for c in range(N_CORES)], axis=1
    )
    return (ct, ht), res


def kernel(**inputs) -> tuple:
    (ct, ht), _ = run(inputs)
    return ct, ht

